# revision 39
# baseline (speedup 1.0000x reference)
"""Trainium2 Bass kernel for a dense transformer block (B=2, T=2048, C=1024,
H=16, Dff=4096), SPMD across 8 NeuronCores.

Sharding: attention is head-parallel (2 heads/core); one AllToAll per batch
redistributes the attention output into a token-parallel layout; projection,
layernorms and the FFN then run on each core's 512-token slice with full
weights, split into 256-token batch-halves so the second AllToAll hides under
the first half's projection+FFN1 work. QKV chunks interleave with attention
j-chunks so the PE-bound QKV stretches cover the scalar-engine exp stretches.
All on-device activations are feature-major (transposed); the host performs
the x -> x^T and out^T -> out transposes during marshalling. The QKV/attention
path runs in bf16; FFN weights are bf16; accumulation is fp32 in PSUM.
"""

import sys

sys.path.insert(0, "/opt/trn_rl_repo")

import numpy as np
import ml_dtypes
import concourse.bacc as bacc
import concourse.mybir as mybir
import concourse.tile as tile
import concourse.bass_utils as bass_utils

try:  # make the NTFF profile shim importable as antenv.axon_hooks
    import antenv

    if "/opt/trn_rl_repo/antenv" not in antenv.__path__:
        antenv.__path__.append("/opt/trn_rl_repo/antenv")
except Exception:
    pass

f32 = mybir.dt.float32
f32r = mybir.dt.float32r
bf16 = mybir.dt.bfloat16
i16 = mybir.dt.int16
AF = mybir.ActivationFunctionType
ALU = mybir.AluOpType

# Schraudolph-style exp for the second head: int16(x*A + B) reinterpreted as
# bf16 approximates exp(0.125*x) to ~±4%. The mask folds in via the bias
# table: masked columns use bias 2000, making the result a tiny positive
# denormal (~1e-33) instead of exp.
SCHR_A = 0.125 * 128.0 / float(np.log(2.0))
SCHR_B = 16248.9
SCHR_MASKED_B = 2000.0

NC = 8          # cores
B = 2           # batch
T = 2048        # sequence length
C = 1024        # model dim
H = 16          # heads
HD = 64         # head dim
HPC = H // NC   # heads per core (2)
DH = HPC * HD   # per-core head cols (128)
DFF = 4096
TOK = B * T     # 4096 tokens
TOKC = TOK // NC  # 512 tokens per core
HT = TOKC // B  # 256 tokens per batch per core
CT = C // 128   # 8 c-tiles
FT = DFF // 128  # 32 ff-tiles
KT = T // 128   # 16 k-tiles per batch
QC = T // 512   # 4 q-chunks of 512 per batch
LN_EPS = 1e-5

_CACHE = {}


def _build(debug=False):
    nc = bacc.Bacc("TRN2", target_bir_lowering=False, debug=False, num_devices=NC)

    # ---- DRAM I/O (per-core values supplied via in_maps) ----
    wq_d = nc.dram_tensor("wq_c", [128, CT, 128], bf16, kind="ExternalInput")
    wk_d = nc.dram_tensor("wk_c", [128, CT, 128], bf16, kind="ExternalInput")
    wv_d = nc.dram_tensor("wv_c", [128, CT, 128], bf16, kind="ExternalInput")
    xt_d = nc.dram_tensor("xt", [C, TOK], bf16, kind="ExternalInput")
    xres_d = nc.dram_tensor("xres_c", [C, TOKC], f32r, kind="ExternalInput")
    wp_d = nc.dram_tensor("wproj", [C, C], bf16, kind="ExternalInput")
    w1_d = nc.dram_tensor("w1p", [FT, 128, CT, 128], bf16, kind="ExternalInput")
    w2_d = nc.dram_tensor("w2", [DFF, C], bf16, kind="ExternalInput")
    bias_d = nc.dram_tensor("biaspack", [128, 6 * CT], f32, kind="ExternalInput")
    b1_d = nc.dram_tensor("b1t", [128, FT], f32, kind="ExternalInput")
    ones_d = nc.dram_tensor("onesp", [128, 128], f32r, kind="ExternalInput")
    identb_d = nc.dram_tensor("identb", [128, 128], bf16, kind="ExternalInput")
    mask_d = nc.dram_tensor("cmask", [128, 4, 512], bf16, kind="ExternalInput")
    bm_d = nc.dram_tensor("bmt", [128, 5, 512], f32, kind="ExternalInput")
    out_d = nc.dram_tensor("out", [C, TOKC], f32, kind="ExternalOutput")

    with tile.TileContext(nc) as tc:
        with (
            nc.allow_low_precision(reason="bf16/float32r matmul inputs"),
            tc.tile_pool(name="const", bufs=1) as p_const,
            tc.tile_pool(name="ln1p", bufs=CT) as p_ln1,
            tc.tile_pool(name="wp", bufs=CT) as p_wp,
            tc.tile_pool(name="act2", bufs=8) as p_act2,
            tc.tile_pool(name="dram", bufs=1, space="DRAM") as p_dram,
        ):
            wp_sb = []
            xres = []
            # two half-AllToAlls: batch-0 shards exchange while batch-1
            # attention still computes. Core c owns tokens
            # [c*256,(c+1)*256) of each batch (512 total).
            a2a_in = [
                p_dram.tile([NC, DH, HT], bf16, tag=f"a2ai{b}", name=f"a2ai{b}")
                for b in range(B)
            ]
            a2a_out = [
                p_dram.tile([NC, DH, HT], bf16, tag=f"a2ao{b}", name=f"a2ao{b}")
                for b in range(B)
            ]

            # ======== phase 1: QKV + attention (head-parallel), QKV chunk n
            # interleaved with attention j=n so ACT-bound softmax stretches
            # overlap PE-bound QKV stretches ========
            with (
                tc.tile_pool(name="attn", bufs=1) as p_attn,
                tc.tile_pool(name="p1c", bufs=1) as p1c,
                tc.tile_pool(name="xt", bufs=24) as p_xt,
                tc.tile_pool(name="qkv", bufs=1) as p_qkv,
                tc.tile_pool(name="es", bufs=3) as p_es,
                tc.tile_pool(name="small", bufs=2) as p_small,
                tc.tile_pool(name="ps1", bufs=6, space="PSUM") as ps1,
            ):
                # phase-1 critical-path DMAs first (lead-in compression)
                wq_sb = p1c.tile([128, CT, 128], bf16, tag="wq")
                wk_sb = p1c.tile([128, CT, 128], bf16, tag="wk")
                wv_sb = p1c.tile([128, CT, 128], bf16, tag="wv")
                nc.sync.dma_start(wq_sb[:], wq_d[:])
                nc.sync.dma_start(wk_sb[:], wk_d[:])
                nc.sync.dma_start(wv_sb[:], wv_d[:])
                identb = p1c.tile([128, 128], bf16, tag="identb")
                nc.sync.dma_start(identb[:], identb_d[:])
                masks = p1c.tile([128, 4, 512], bf16, tag="masks")
                nc.sync.dma_start(masks[:], mask_d[:])
                bmt = p1c.tile([128, 5, 512], f32, tag="bmt")
                nc.sync.dma_start(bmt[:], bm_d[:])
                ones = p_const.tile([128, 128], f32r, tag="ones")
                nc.sync.dma_start(ones[:], ones_d[:])
                biasp = p_const.tile([128, 6 * CT], f32, tag="biasp")
                nc.sync.dma_start(biasp[:], bias_d[:])
                b1t = p_const.tile([128, FT], f32, tag="b1t")
                nc.sync.dma_start(b1t[:], b1_d[:])
                onesb = p_const.tile([128, 128], bf16, tag="onesb")
                nc.vector.tensor_copy(onesb[:], ones[:].bitcast(f32))
                # bias pack columns: [bproj | b2 | g1 | be1 | g2 | be2]
                bproj_b = biasp[:, 0 * CT:1 * CT]
                b2_b = biasp[:, 1 * CT:2 * CT]
                g1_b = biasp[:, 2 * CT:3 * CT]
                be1_b = biasp[:, 3 * CT:4 * CT]
                g2_b = biasp[:, 4 * CT:5 * CT]
                be2_b = biasp[:, 5 * CT:6 * CT]

                # per-head attention outputs (feature-major rows 0-63)
                attnh = [
                    p_attn.tile([HD, TOK], bf16, tag=f"attn{h}", name=f"attnh{h}")
                    for h in range(HPC)
                ]

                SL = 132  # vt slot: [Vh0|ones|pad|Vh1|ones|pad]

                def emit_renorm(b, j, oacc):
                    """Softmax renormalize j's accumulators + stage A2A shards."""
                    for h in range(HPC):
                        # free the PSUM accum early via two same-base copies
                        osb = p_small.tile([64, 512], f32, tag="osb", bufs=2)
                        nc.vector.tensor_copy(osb[:], oacc[h][0:64, :])
                        sr = p_small.tile([128, 512], f32r, tag="sr")
                        nc.vector.tensor_copy(sr[64:65, :], oacc[h][64:65, :])
                        # broadcast sums across partitions, then approx-recip
                        bps = ps1.tile([64, 512], f32, tag="oacc", bufs=2)
                        nc.tensor.matmul(
                            bps[:], ones[64:65, 0:64], sr[64:65, :],
                            start=True, stop=True, tile_position=(64, 0),
                        )
                        ibc = p_small.tile([64, 512], f32, tag="ibc")
                        nc.vector.reciprocal_approx_fast(ibc[:], bps[:])
                        nc.vector.tensor_mul(
                            attnh[h][:, b * T + j * 512:b * T + (j + 1) * 512],
                            osb[:],
                            ibc[:],
                        )
                    # stage this chunk's two A2A shards immediately
                    for s in (2 * j, 2 * j + 1):
                        for h in range(HPC):
                            nc.sync.dma_start(
                                a2a_in[b][s, h * 64:(h + 1) * 64, :],
                                attnh[h][:, b * T + s * HT:b * T + (s + 1) * HT],
                            )

                def emit_collective(b):
                    # the b=0 exchange overlaps batch-1 QKV+attention, the
                    # b=1 exchange overlaps the batch-0 proj/LN1/FFN1 chain
                    nc.gpsimd.collective_compute(
                        "AllToAll",
                        ALU.bypass,
                        replica_groups=[list(range(NC))],
                        ins=[a2a_in[b][:].opt()],
                        outs=[a2a_out[b][:].opt()],
                    )
                    if b == 0:
                        # issue the proj/residual weight loads here so they
                        # stream during the remaining batch-1 attention and
                        # complete before the phase-2 start
                        for kt in range(CT):
                            t = p_wp.tile([128, C], bf16, tag="wp")
                            nc.sync.dma_start(
                                t[:], wp_d[kt * 128:(kt + 1) * 128, :]
                            )
                            wp_sb.append(t)
                        for ct in range(CT):
                            t = p_act2.tile([128, TOKC], f32r, tag="xres")
                            nc.sync.dma_start(
                                t[:], xres_d[ct * 128:(ct + 1) * 128, :]
                            )
                            xres.append(t)

                for b in range(B):
                    qT = p_qkv.tile([DH, T], bf16, tag="q")
                    kT = p_qkv.tile([DH, T], bf16, tag="k")
                    vT = p_qkv.tile([DH, T], bf16, tag="v")
                    vt = p_qkv.tile([128, KT, SL], bf16, tag="vt")

                    for n in range(QC):
                        # ---- QKV projections for chunk n (feature-major) ----
                        ncol = slice(n * 512, (n + 1) * 512)
                        xt_sb = []
                        for ct in range(CT):
                            t = p_xt.tile([128, 512], bf16, tag="xt", bufs=24)
                            nc.sync.dma_start(
                                t[:],
                                xt_d[ct * 128:(ct + 1) * 128,
                                     b * T + n * 512:b * T + (n + 1) * 512],
                            )
                            xt_sb.append(t)
                        pq = ps1.tile([128, 512], f32, tag="sps", bufs=3)
                        for ct in range(CT):
                            nc.tensor.matmul(
                                pq[:], wq_sb[:, ct, :], xt_sb[ct][:],
                                start=(ct == 0), stop=(ct == CT - 1),
                            )
                        nc.vector.tensor_copy(qT[:, ncol], pq[:])
                        pk = ps1.tile([128, 512], f32, tag="sps", bufs=3)
                        for ct in range(CT):
                            nc.tensor.matmul(
                                pk[:], wk_sb[:, ct, :], xt_sb[ct][:],
                                start=(ct == 0), stop=(ct == CT - 1),
                            )
                        nc.vector.tensor_copy(kT[:, ncol], pk[:])
                        pv = ps1.tile([128, 512], f32, tag="sps", bufs=3)
                        for ct in range(CT):
                            nc.tensor.matmul(
                                pv[:], wv_sb[:, ct, :], xt_sb[ct][:],
                                start=(ct == 0), stop=(ct == CT - 1),
                            )
                        nc.vector.tensor_copy(vT[:, ncol], pv[:])

                        j = n
                        nkt = 4 * j + 4

                        def emit_score(kt, j=j, qT=qT, kT=kT):
                            """Score pair matmuls + exp for one k-tile."""
                            m = kt - 4 * j
                            spair = ps1.tile([128, 2, 512], f32, tag="sps",
                                             bufs=3)
                            for h in range(HPC):
                                hrow = slice(h * 64, (h + 1) * 64)
                                nc.tensor.matmul(
                                    spair[:, h, :],
                                    kT[hrow, kt * 128:(kt + 1) * 128],
                                    qT[hrow, j * 512:(j + 1) * 512],
                                    start=True, stop=True,
                                    tile_position=(64 * h, 0),
                                )
                            epair = p_es.tile([128, 2, 512], bf16, tag="es",
                                              bufs=8)
                            # h1: Schraudolph exp on DVE (mask folded into
                            # the bias table); h0: exact exp on ACT
                            nc.vector.scalar_tensor_tensor(
                                epair[:, 1, :].bitcast(i16),
                                spair[:, 1, :], SCHR_A,
                                bmt[:, 0 if m < 0 else 1 + m, :],
                                ALU.mult, ALU.add,
                            )
                            if m < 0:
                                nc.scalar.activation(
                                    epair[:, 0, :], spair[:, 0, :],
                                    AF.Exp, scale=0.125,
                                )
                            else:
                                ed = p_es.tile([128, 512], bf16, tag="esd",
                                               bufs=3)
                                nc.scalar.activation(
                                    ed[:], spair[:, 0, :], AF.Exp, scale=0.125
                                )
                                nc.vector.tensor_mul(
                                    epair[:, 0, :], ed[:], masks[:, m, :]
                                )
                            return epair

                        # prime the first two k-tiles' scores so their exps
                        # run under the V-transpose block and the first attnV
                        # never waits on exp latency
                        eprimed = {kt: emit_score(kt) for kt in range(2)}

                        # ---- V -> token-major for this chunk's 4 k-tiles ----
                        for kt in range(4 * n, 4 * n + 4):
                            pt = ps1.tile([128, 128], bf16, tag="oacc", bufs=2)
                            nc.tensor.transpose(
                                pt[:], vT[:, kt * 128:(kt + 1) * 128], identb[:]
                            )
                            nc.vector.tensor_copy(vt[:, kt, 0:64], pt[:, 0:64])
                            nc.vector.tensor_copy(vt[:, kt, 66:130], pt[:, 64:128])
                            nc.vector.tensor_copy(vt[:, kt, 64:65], onesb[:, 0:1])
                            nc.vector.tensor_copy(vt[:, kt, 130:131], onesb[:, 1:2])

                        # ---- causal attention j = n: both heads interleaved
                        # per k-tile (score matmuls pack into disjoint rows) ----
                        oacc = [
                            ps1.tile([65, 512], f32, tag="oacc", bufs=2,
                                     name=f"oacc{h}")
                            for h in range(HPC)
                        ]
                        for kt in range(nkt):
                            epair = eprimed.pop(kt, None)
                            if epair is None:
                                epair = emit_score(kt)
                            for h in range(HPC):
                                nc.tensor.matmul(
                                    oacc[h][:],
                                    vt[:, kt, 66 * h:66 * h + 65],
                                    epair[:, h, :],
                                    start=(kt == 0), stop=(kt == nkt - 1),
                                )
                        emit_renorm(b, j, oacc)

                    emit_collective(b)

            # ======== phase 2+3: proj + LN1 + FFN + LN2, per batch-half ======
            with (
                tc.tile_pool(name="w2s", bufs=6) as p_w2,
                tc.tile_pool(name="w1s", bufs=8) as p_w1,
                tc.tile_pool(name="hff", bufs=FT) as p_hff,
                tc.tile_pool(name="agg", bufs=8) as p_agg,
                tc.tile_pool(name="tmp2", bufs=2) as p_tmp2,
                tc.tile_pool(name="outp", bufs=4) as p_out,
                tc.tile_pool(name="ps2", bufs=4, space="PSUM") as ps2,
            ):
                x1 = [
                    p_act2.tile([128, TOKC], bf16, tag="x1", name=f"x1_{i}")
                    for i in range(CT)
                ]
                ln1 = [
                    p_ln1.tile([128, TOKC], bf16, tag="ln1", name=f"ln1_{i}")
                    for i in range(CT)
                ]

                def ln_stats_finish(s1, s2, nh, x_of, out_slice, g_b, be_b,
                                    post=None):
                    """Given accumulated s1/s2 PSUM rows, normalize x tiles."""
                    nmu = p_tmp2.tile([1, nh], f32r, tag=f"nmu{nh}")
                    nc.vector.tensor_scalar_mul(nmu[:], s1[:], -1.0 / C)
                    ex2 = p_tmp2.tile([1, nh], f32, tag=f"ex2{nh}")
                    nc.vector.tensor_scalar_mul(ex2[:], s2[:], 1.0 / C)
                    mu2 = p_tmp2.tile([1, nh], f32, tag=f"mu2{nh}")
                    nc.vector.tensor_mul(
                        mu2[:], nmu[:].bitcast(f32), nmu[:].bitcast(f32)
                    )
                    var = p_tmp2.tile([1, nh], f32, tag=f"var{nh}")
                    nc.vector.tensor_sub(var[:], ex2[:], mu2[:])
                    nc.vector.tensor_scalar_add(var[:], var[:], LN_EPS)
                    sd = p_tmp2.tile([1, nh], f32r, tag=f"sd{nh}")
                    nc.scalar.activation(sd[:], var[:], AF.Sqrt, bias=0.0)
                    bmu = ps2.tile([128, nh], f32, tag="ln", bufs=2)
                    nc.tensor.matmul(
                        bmu[:], ones[0:1, :], nmu[:], start=True, stop=True
                    )
                    brs = ps2.tile([128, nh], f32, tag="ln", bufs=2)
                    nc.tensor.matmul(
                        brs[:], ones[0:1, :], sd[:], start=True, stop=True
                    )
                    bmu_sb = p_tmp2.tile([128, nh], f32, tag=f"bmu{nh}")
                    nc.vector.tensor_copy(bmu_sb[:], bmu[:])
                    brs_sb = p_tmp2.tile([128, nh], f32, tag=f"brs{nh}")
                    nc.vector.reciprocal_approx_fast(brs_sb[:], brs[:])
                    for ct in range(CT):
                        t1 = p_tmp2.tile([128, nh], f32, tag=f"lntmp{nh}")
                        nc.vector.tensor_add(t1[:], x_of(ct), bmu_sb[:])
                        t2 = p_tmp2.tile([128, nh], f32, tag=f"lntmp2{nh}")
                        nc.vector.tensor_mul(t2[:], t1[:], brs_sb[:])
                        nc.scalar.activation(
                            out_slice(ct), t2[:], AF.Identity,
                            bias=be_b[:, ct:ct + 1], scale=g_b[:, ct:ct + 1],
                        )
                        if post is not None:
                            post(ct)

                for hb in range(B):
                    cols = slice(hb * HT, (hb + 1) * HT)
                    # gather this half's attention tokens (all 16 heads)
                    ag = []
                    for kt in range(CT):
                        t = p_agg.tile([128, HT], bf16, tag="ag",
                                       name=f"ag{hb}_{kt}")
                        nc.sync.dma_start(t[:], a2a_out[hb][kt])
                        ag.append(t)
                    # ---- proj + residual, LN1 stats interleaved per mt ----
                    s1 = ps2.tile([1, HT], f32, tag="ln", bufs=2)
                    s2 = ps2.tile([1, HT], f32, tag="ln", bufs=2)
                    for mt in range(CT):
                        yps = ps2.tile([128, HT], f32, tag="yps", bufs=2)
                        for kt in range(CT):
                            nc.tensor.matmul(
                                yps[:],
                                wp_sb[kt][:, mt * 128:(mt + 1) * 128],
                                ag[kt][:],
                                start=(kt == 0), stop=(kt == CT - 1),
                            )
                        t1 = p_tmp2.tile([128, HT], f32, tag="projt")
                        nc.scalar.activation(
                            t1[:], yps[:], AF.Identity,
                            bias=bproj_b[:, mt:mt + 1],
                        )
                        nc.vector.tensor_add(
                            x1[mt][:, cols], t1[:],
                            xres[mt][:, cols].bitcast(f32),
                        )
                        nc.tensor.matmul(
                            s1[:], onesb[:, 0:1], x1[mt][:, cols],
                            start=(mt == 0), stop=(mt == CT - 1),
                        )
                        sq = p_tmp2.tile([128, HT], bf16, tag="sq1")
                        nc.vector.tensor_mul(
                            sq[:], x1[mt][:, cols], x1[mt][:, cols]
                        )
                        nc.tensor.matmul(
                            s2[:], onesb[:, 0:1], sq[:],
                            start=(mt == 0), stop=(mt == CT - 1),
                        )
                    ln_stats_finish(
                        s1, s2, HT, lambda ct: x1[ct][:, cols],
                        lambda ct: ln1[ct][:, cols], g1_b, be1_b,
                    )

                    # ---- FFN1: h = relu(w1^T @ ln1 + b1) ----
                    hff = []
                    for mt in range(FT):
                        w1t = p_w1.tile([128, CT, 128], bf16, tag="w1")
                        nc.sync.dma_start(w1t[:], w1_d[mt])
                        yps = ps2.tile([128, HT], f32, tag="yps", bufs=2)
                        for kt in range(CT):
                            nc.tensor.matmul(
                                yps[:], w1t[:, kt, :], ln1[kt][:, cols],
                                start=(kt == 0), stop=(kt == CT - 1),
                            )
                        hf = p_hff.tile([128, HT], bf16, tag="hff",
                                        name=f"hff{hb}_{mt}")
                        nc.scalar.activation(
                            hf[:], yps[:], AF.Relu, bias=b1t[:, mt:mt + 1]
                        )
                        hff.append(hf)

                    # ---- FFN2 kt-outer (streamed w2); two mt accumulators
                    # share one PSUM bank (single start=True clears the bank,
                    # the sibling slice first-writes with bits clear) ----
                    accp = [
                        ps2.tile([128, 2, HT], f32, tag="acc", bufs=4,
                                 name=f"accp{hb}_{mi}")
                        for mi in range(CT // 2)
                    ]
                    for kt in range(FT):
                        w2t = p_w2.tile([128, C], bf16, tag="w2")
                        nc.sync.dma_start(
                            w2t[:], w2_d[kt * 128:(kt + 1) * 128, :]
                        )
                        for mi in range(CT // 2):
                            for sl in range(2):
                                mt = 2 * mi + sl
                                nc.tensor.matmul(
                                    accp[mi][:, sl, :],
                                    w2t[:, mt * 128:(mt + 1) * 128],
                                    hff[kt][:],
                                    start=(kt == 0 and sl == 0),
                                    stop=(kt == FT - 1),
                                    skip_group_check=True,
                                )
                    t1s = ps2.tile([1, HT], f32, tag="ln", bufs=2)
                    t2s = ps2.tile([1, HT], f32, tag="ln", bufs=2)
                    x2 = []
                    for mt in range(CT):
                        mi, sl = divmod(mt, 2)
                        t1 = p_tmp2.tile([128, HT], f32, tag="ffn2t")
                        nc.scalar.activation(
                            t1[:], accp[mi][:, sl, :], AF.Identity,
                            bias=b2_b[:, mt:mt + 1],
                        )
                        xr = p_act2.tile([128, HT], f32r, tag="x2",
                                         name=f"x2_{hb}_{mt}")
                        nc.vector.tensor_add(xr[:], t1[:], ln1[mt][:, cols])
                        x2.append(xr)
                        nc.tensor.matmul(
                            t1s[:], ones[:, 0:1], xr[:],
                            start=(mt == 0), stop=(mt == CT - 1),
                        )
                        sq = p_tmp2.tile([128, HT], bf16, tag="sq2")
                        nc.vector.tensor_mul(
                            sq[:], xr[:].bitcast(f32), xr[:].bitcast(f32)
                        )
                        nc.tensor.matmul(
                            t2s[:], onesb[:, 0:1], sq[:],
                            start=(mt == 0), stop=(mt == CT - 1),
                        )

                    out_tiles = {}

                    def store_out(ct):
                        t = p_out.tile([128, HT], f32, tag="outt")
                        out_tiles[ct] = t
                        return t[:, 0:HT]

                    def dma_out(ct, cols=cols):
                        nc.sync.dma_start(
                            out_d[ct * 128:(ct + 1) * 128, cols],
                            out_tiles[ct][:],
                        )

                    ln_stats_finish(
                        t1s, t2s, HT, lambda ct: x2[ct][:].bitcast(f32),
                        store_out, g2_b, be2_b, post=dma_out,
                    )

    nc.compile()
    return nc


def _pack_inputs(inputs):
    """Host-side sharding/marshalling. Returns in_maps for the 8 cores."""
    x = np.asarray(inputs["x"], dtype=np.float32)
    xf = np.ascontiguousarray(x.reshape(TOK, C))
    xt32 = np.ascontiguousarray(xf.T)  # [C, TOK]
    xt = np.ascontiguousarray(xt32.astype(ml_dtypes.bfloat16))
    wq = np.asarray(inputs["wq"], dtype=np.float32)
    wk = np.asarray(inputs["wk"], dtype=np.float32)
    wv = np.asarray(inputs["wv"], dtype=np.float32)
    wproj = np.ascontiguousarray(
        np.asarray(inputs["w_proj"], dtype=np.float32).astype(ml_dtypes.bfloat16)
    )
    w1 = np.asarray(inputs["w1"], dtype=np.float32)
    w2 = np.ascontiguousarray(
        np.asarray(inputs["w2"], dtype=np.float32).astype(ml_dtypes.bfloat16)
    )
    # w1 packed per ff-tile: [FT, 128(p), CT, 128(f)];  w1 is [C, DFF]
    w1p = np.ascontiguousarray(
        w1.reshape(CT, 128, FT, 128).transpose(2, 1, 0, 3).astype(ml_dtypes.bfloat16)
    )

    def tile_vec(v, n):
        return np.ascontiguousarray(
            np.asarray(v, dtype=np.float32).reshape(n, 128).T
        )

    biaspack = np.zeros((128, 6 * CT), dtype=np.float32)
    biaspack[:, 0 * CT:1 * CT] = tile_vec(inputs["b_proj"], CT)
    biaspack[:, 1 * CT:2 * CT] = tile_vec(inputs["b2"], CT)
    biaspack[:, 2 * CT:3 * CT] = tile_vec(inputs["g1"], CT)
    biaspack[:, 3 * CT:4 * CT] = tile_vec(inputs["be1"], CT)
    biaspack[:, 4 * CT:5 * CT] = tile_vec(inputs["g2"], CT)
    biaspack[:, 5 * CT:6 * CT] = tile_vec(inputs["be2"], CT)
    b1t = tile_vec(inputs["b1"], FT)

    # causal masks for the 4 diagonal offsets, packed [128, 4, 512]
    r = np.arange(128)[:, None]
    ccol = np.arange(512)[None, :]
    cmask = np.stack(
        [(ccol >= r + 128 * m).astype(np.float32) for m in range(4)], axis=1
    )
    cmask = np.ascontiguousarray(cmask).astype(ml_dtypes.bfloat16)
    # Schraudolph bias table: [:,0,:] = B everywhere (off-diagonal k-tiles);
    # [:,1+m,:] = B where causally visible, small positive bias where masked
    cmask_f = np.stack(
        [(ccol >= r + 128 * m) for m in range(4)], axis=1
    )  # [128, 4, 512] bool
    bmt = np.empty((128, 5, 512), dtype=np.float32)
    bmt[:, 0, :] = SCHR_B
    bmt[:, 1:, :] = np.where(cmask_f, SCHR_B, SCHR_MASKED_B)
    bmt = np.ascontiguousarray(bmt)
    onesp = np.ones((128, 128), dtype=np.float32)
    identb = np.eye(128, dtype=np.float32).astype(ml_dtypes.bfloat16)

    in_maps = []
    for c in range(NC):
        hcol = slice(c * DH, (c + 1) * DH)

        def pack_w(w):
            return np.ascontiguousarray(
                w[:, hcol].reshape(CT, 128, DH).transpose(1, 0, 2)
                .astype(ml_dtypes.bfloat16)
            )

        in_maps.append(
            {
                "xt": xt,
                "wq_c": pack_w(wq),
                "wk_c": pack_w(wk),
                "wv_c": pack_w(wv),
                "xres_c": np.ascontiguousarray(
                    np.concatenate(
                        [
                            xt32[:, b * T + c * HT:b * T + (c + 1) * HT]
                            for b in range(B)
                        ],
                        axis=1,
                    )
                ),
                "wproj": wproj,
                "w1p": w1p,
                "w2": w2,
                "biaspack": biaspack,
                "b1t": b1t,
                "onesp": onesp,
                "identb": identb,
                "cmask": cmask,
                "bmt": bmt,
            }
        )
    return in_maps


def _run(inputs, trace=False, debug=False):
    if "rel" not in _CACHE:
        _CACHE["rel"] = _build()
    nc = _CACHE["rel"]
    in_maps = _pack_inputs(inputs)
    res = bass_utils.run_bass_kernel_spmd(
        nc, in_maps, core_ids=list(range(NC)), trace=trace
    )
    out = np.empty((TOK, C), dtype=np.float32)
    for c in range(NC):
        oc = res.results[c]["out"]
        for b in range(B):
            out[b * T + c * HT:b * T + (c + 1) * HT, :] = (
                oc[:, b * HT:(b + 1) * HT].T
            )
    return out.reshape(B, T, C), res


def kernel(**inputs) -> np.ndarray:
    out, _ = _run(inputs, trace=False, debug=False)
    return out
